# revision 7
# baseline (speedup 1.0000x reference)
"""Trainium2 Bass kernel for AttnBlock (GroupNorm + 1x1-conv QKV self-attention
+ output proj + residual) on x: [4, 512, 64, 64] fp32, distributed over 8
NeuronCores.

Sharding: data-parallel over batch (4) x sequence-parallel over the N=H*W=4096
token axis (2 halves) = 8 cores. Each core receives the full image of its
batch element with the token axis rotated so that its 2048 query tokens come
first; it computes GroupNorm + K/V for all 4096 tokens (duplicated within the
batch pair -- no collectives needed) and Q/attention/output only for its 2048
queries. The host gathers the 8 [512, 2048] outputs back into [4, 512, 64, 64].

All large matmuls run in fp8e4 with MatmulPerfMode.DoubleRow (2 contraction
k-tiles per instruction, ~2x bf16 PE throughput) and fp32 PSUM accumulation;
only the O-projection stays bf16 (its operand, the unnormalized attention
output, exceeds fp8e4's +-240 range). Softmax runs in fp32 (exp on the scalar
engine straight out of PSUM, with a constant -1.5 shift so the fp8 exp output
stays below the 240 saturation point; the shift cancels in the softmax ratio).
Structure:
- x ships once in bf16; the scalar engine casts it to fp8 pair-layout tiles
  while the vector engine runs GroupNorm stats (bn_stats at 2x 16-bit rate)
  on the same bf16 chunks as they stream in.
- GroupNorm is folded into the projections: wk@(s*x+t) = (wk*s)@x + (wk@t),
  so K/Q/V matmuls consume raw fp8 x tiles; the per-channel scale s lands in
  the fp8 weights (tiny DVE ops) and wk@t lands in the biases (tiny PE
  matmuls). 1/sqrt(var+eps) is computed with a Newton iteration on the
  vector engine so the scalar engine only ever runs Identity/Exp -- a single
  activation-table load for the whole kernel (table-set switches cost ~2.7us).
- All fp8 operands use a channel/key-paired [128, 2, *] layout so each
  DoubleRow matmul consumes two 128-wide contraction tiles at once.
- Scores are computed transposed (S^T = K^T Q per key tile) so softmax and
  the attention@V contraction need no transposes at all.
- The softmax denominator is accumulated on GpSimd for the first half of the
  key tiles and DVE for the second, then column-reduced by two accumulating
  PE matmuls -- the GpSimd half issues mid-block so only the DVE half is on
  the block-boundary critical path. The 1/denominator is applied after the
  O-projection (it commutes with the linear projection).
- Score/exp lookahead into the next query block is interleaved with the
  denominator/reciprocal tail so the PE never drains at block boundaries;
  residual tiles (x+bo) prefetch one block ahead; the final normalize+residual
  ops are split DVE/GpSimd.
"""

import numpy as np
import ml_dtypes

B, C, H, W = 4, 512, 64, 64
N = H * W            # 4096 tokens
NQ = N // 2          # 2048 queries per core
P = 128              # partitions
CT = C // P          # 4 channel tiles
CP = CT // 2         # 2 channel pair-tiles (DoubleRow)
JT = N // P          # 32 key/token tiles
JP = JT // 2         # 16 key pair-tiles (DoubleRow)
IBS = 512            # query block (free dim of score matmuls)
IB = NQ // IBS       # 4 query blocks per core
NCH = N // IBS       # 8 n-chunks for full-N projections
GROUPS = 32
GSIZE = C // GROUPS  # 16 channels per group
EPS = 1e-6
SM_SCALE = float(C) ** -0.5
EXP_SHIFT = -1.5     # exp(s + shift): keeps fp8 exp output < 240 saturation

N_CORES = 8

_cache = {}


def _build_nc():
    import concourse.bass as bass
    import concourse.mybir as mybir
    import concourse.tile as tile
    from concourse import bacc

    f32 = mybir.dt.float32
    bf16 = mybir.dt.bfloat16
    fp8 = mybir.dt.float8e4
    DR = mybir.MatmulPerfMode.DoubleRow
    ID = mybir.ActivationFunctionType.Identity
    EXP = mybir.ActivationFunctionType.Exp

    nc = bacc.Bacc("TRN2")

    xs_d = nc.declare_dram_parameter("xs", [C, N], bf16, isOutput=False)
    w_d = {
        name: nc.declare_dram_parameter(name, [C, C], bf16, isOutput=False)
        for name in ("wqT", "wkT", "wvT", "woT")
    }
    cols_d = nc.declare_dram_parameter("cols", [C, 6], f32, isOutput=False)
    xqb_d = nc.declare_dram_parameter("xqb", [C, NQ], f32, isOutput=False)
    inda_d = nc.declare_dram_parameter("ind_a", [P, CT * GROUPS], bf16, isOutput=False)
    indb_d = nc.declare_dram_parameter("ind_b", [GROUPS, CT * P], bf16, isOutput=False)
    out_d = nc.declare_dram_parameter("out", [C, NQ], f32, isOutput=True)

    with tile.TileContext(nc) as tc:
        from contextlib import ExitStack

        with ExitStack() as ctx:
            const = ctx.enter_context(tc.tile_pool(name="const", bufs=1))
            pp_mm = ctx.enter_context(tc.tile_pool(name="pp_mm", bufs=3, space="PSUM"))
            pp_av = ctx.enter_context(tc.tile_pool(name="pp_av", bufs=4, space="PSUM"))
            pp_sm = ctx.enter_context(tc.tile_pool(name="pp_sm", bufs=1, space="PSUM"))

            # ---- batched small constants (few DMAs; issued after x) ----
            cols_t = [const.tile([P, 6], f32, tag=f"cols{t}", name=f"cols{t}")
                      for t in range(CT)]
            inda_t = const.tile([P, CT * GROUPS], bf16, tag="inda", name="inda")
            indb_t = const.tile([GROUPS, CT * P], bf16, tag="indb", name="indb")
            col_sb = {nm: [cols_t[t][:, i:i + 1] for t in range(CT)]
                      for i, nm in enumerate(("bq", "bk", "bv", "bo",
                                              "gamma", "beta"))}
            inda_sb = [inda_t[:, t * GROUPS:(t + 1) * GROUPS] for t in range(CT)]
            indb_sb = [indb_t[:, t * P:(t + 1) * P] for t in range(CT)]

            ones_colf = const.tile([P, 1], f32, tag="ones_colf", name="ones_colf")
            nc.vector.memset(ones_colf, 1.0)
            ones_rowf = const.tile([1, P], f32, tag="ones_rowf", name="ones_rowf")
            nc.vector.memset(ones_rowf, 1.0)
            shift_col = const.tile([P, 1], f32, tag="shift_col", name="shift_col")
            nc.vector.memset(shift_col, EXP_SHIFT)
            # touch Exp immediately so the one activation-table load (the set
            # holding Exp; Identity is filler in every set) happens at kernel
            # start instead of stalling the first attention block
            warm = const.tile([1, 1], f32, tag="warm", name="warm")
            nc.scalar.activation(out=warm, in_=shift_col[0:1, 0:1], func=EXP)

            stat_pool = ctx.enter_context(tc.tile_pool(name="stat", bufs=4 * CT))

            k_pool = ctx.enter_context(tc.tile_pool(name="k", bufs=CP))
            v_pool = ctx.enter_context(tc.tile_pool(name="v", bufs=JP))
            q_pool = ctx.enter_context(tc.tile_pool(name="q", bufs=CP))
            k2_sb = [k_pool.tile([P, 2, N], fp8, tag="k", name="k")
                     for _ in range(CP)]
            q2_sb = [q_pool.tile([P, 2, NQ], fp8, tag="q", name="q")
                     for _ in range(CP)]

            # ---- phase 1: x load (2 HW-DGE queues) + GroupNorm stats on DVE
            # (bf16 bn_stats at 2x rate) + fp8 pair-layout cast on ACT ----
            mv_sb = []
            with tc.tile_pool(name="xr", bufs=CP) as xr_pool, \
                 tc.tile_pool(name="xs", bufs=CT) as xs_pool:
                x2_sb = [xr_pool.tile([P, 2, N], fp8, tag="xr", name="xr")
                         for _ in range(CP)]
                xs_sb = [xs_pool.tile([P, N], bf16, tag="xs", name="xs")
                         for _ in range(CT)]
                CHW = N // 4
                # chunk-major load order so early columns of every tile land
                # first (casts + first projections consume column-blocks)
                for ch in range(4):
                    for t in range(CT):
                        eng = nc.sync if (ch * CT + t) % 2 == 0 else nc.scalar
                        csl = slice(ch * CHW, (ch + 1) * CHW)
                        eng.dma_start(out=xs_sb[t][:, csl],
                                      in_=xs_d[t * P:(t + 1) * P, csl])
                # fp8 cast on ACT, chunk-major; DVE bn_stats per 512-chunk
                st_sb = [stat_pool.tile([P, N // 512, 6], f32, tag=f"bnst{t}",
                                        name=f"bnst{t}") for t in range(CT)]
                for ch in range(4):
                    for t in range(CT):
                        csl = slice(ch * CHW, (ch + 1) * CHW)
                        nc.scalar.activation(out=x2_sb[t // 2][:, t % 2, csl],
                                             in_=xs_sb[t][:, csl], func=ID)
                        for s in range(2 * ch, 2 * ch + 2):
                            nc.vector.bn_stats(
                                out=st_sb[t][:, s, :],
                                in_=xs_sb[t][:, s * 512:(s + 1) * 512])

                # batched consts + weights + bv now (queues free after x)
                nc.sync.dma_start(out=inda_t, in_=inda_d[:, :])
                nc.sync.dma_start(out=indb_t, in_=indb_d[:, :])
                for t in range(CT):
                    nc.sync.dma_start(out=cols_t[t],
                                      in_=cols_d[t * P:(t + 1) * P, :])
                worig_cm = tc.tile_pool(name="worig", bufs=1)
                worig_pool = worig_cm.__enter__()
                w_sb = {}
                for name in ("wkT", "wqT", "wvT", "woT"):
                    tiles = []
                    for t in range(CT):
                        pool = const if name == "woT" else worig_pool
                        tw = pool.tile([P, C], bf16, tag=f"{name}{t}",
                                       name=f"{name}{t}")
                        nc.sync.dma_start(out=tw,
                                          in_=w_d[name][t * P:(t + 1) * P, :])
                        tiles.append(tw)
                    w_sb[name] = tiles
                bv_row = const.tile([1, C], f32, tag="bv_row", name="bv_row")
                nc.sync.dma_start(
                    out=bv_row,
                    in_=cols_d[:, 2:3].rearrange("c one -> one c"))

                for t in range(CT):
                    mv = stat_pool.tile([P, 2], f32, tag="mv", name="mv")
                    nc.vector.bn_aggr(out=mv, in_=st_sb[t])
                    # mv = [mean, var] -> [mean, E[x^2]]
                    msq = stat_pool.tile([P, 1], f32, tag="msq", name="msq")
                    nc.vector.tensor_mul(msq, mv[:, 0:1], mv[:, 0:1])
                    nc.vector.tensor_add(mv[:, 1:2], mv[:, 1:2], msq)
                    mvb = stat_pool.tile([P, 2], bf16, tag="mvb", name="mvb")
                    nc.vector.tensor_copy(out=mvb, in_=mv)
                    mv_sb.append(mvb)

                # aggregate over channel groups: [32, 2] = [mean_g, E[x^2]_g]
                g_ps = pp_sm.tile([GROUPS, 2], f32, tag="den", name="den")
                for t in range(CT):
                    nc.tensor.matmul(g_ps, lhsT=inda_sb[t], rhs=mv_sb[t],
                                     start=(t == 0), stop=(t == CT - 1))
                g_sb = stat_pool.tile([GROUPS, 2], f32, tag="gsb", name="gsb")
                nc.vector.tensor_copy(out=g_sb, in_=g_ps)
                gm2 = stat_pool.tile([GROUPS, 1], f32, tag="gm2", name="gm2")
                nc.vector.tensor_mul(gm2, g_sb[:, 0:1], g_sb[:, 0:1])
                gvar = stat_pool.tile([GROUPS, 1], f32, tag="gvar", name="gvar")
                nc.vector.tensor_sub(gvar, g_sb[:, 1:2], gm2)
                # ga = 1/sqrt(gvar+EPS) on DVE: y0 = 1.5 - 0.5 v (var ~= 1
                # after GroupNorm-scale inputs), then two Newton steps
                # y <- y(1.5 - 0.5 v y^2). Avoids the ACT Sqrt table set.
                veps = stat_pool.tile([GROUPS, 1], f32, tag="veps", name="veps")
                nc.vector.tensor_scalar(out=veps, in0=gvar, scalar1=EPS,
                                        scalar2=None, op0=mybir.AluOpType.add)
                ga = stat_pool.tile([GROUPS, 1], f32, tag="ga", name="ga")
                nc.vector.tensor_scalar(out=ga, in0=veps, scalar1=-0.5,
                                        scalar2=1.5, op0=mybir.AluOpType.mult,
                                        op1=mybir.AluOpType.add)
                for _ in range(2):
                    yy = stat_pool.tile([GROUPS, 1], f32, tag="yy", name="yy")
                    nc.vector.tensor_mul(yy, ga, ga)
                    nc.vector.tensor_mul(yy, veps, yy)
                    nc.vector.tensor_scalar(out=yy, in0=yy, scalar1=-0.5,
                                            scalar2=1.5,
                                            op0=mybir.AluOpType.mult,
                                            op1=mybir.AluOpType.add)
                    nc.vector.tensor_mul(ga, ga, yy)
                coeffs = stat_pool.tile([GROUPS, 2], bf16, tag="coef", name="coef")
                nc.vector.tensor_copy(out=coeffs[:, 0:1], in_=ga)
                nc.vector.tensor_copy(out=coeffs[:, 1:2], in_=g_sb[:, 0:1])

                # broadcast group coeffs to per-channel scale/shift columns
                sc_cols = []
                tc_cols = []
                for t in range(CT):
                    b_ps = pp_sm.tile([P, 2], f32, tag="den", name="den")
                    nc.tensor.matmul(b_ps, lhsT=indb_sb[t], rhs=coeffs,
                                     start=True, stop=True)
                    bc = stat_pool.tile([P, 2], f32, tag="bc", name="bc")
                    nc.vector.tensor_copy(out=bc, in_=b_ps)
                    s_col = stat_pool.tile([P, 1], f32, tag="scol", name="scol")
                    nc.vector.tensor_mul(s_col, col_sb["gamma"][t], bc[:, 0:1])
                    tmp = stat_pool.tile([P, 1], f32, tag="tmp", name="tmp")
                    nc.vector.tensor_mul(tmp, bc[:, 1:2], s_col)
                    t_col = stat_pool.tile([P, 1], f32, tag="tcol", name="tcol")
                    nc.vector.tensor_sub(t_col, col_sb["beta"][t], tmp)
                    sc_cols.append(s_col)
                    tc_cols.append(t_col)

                # GroupNorm folding: wk@(s*x+t) = (wk*s)@x + wk@t.  Scale the
                # QKV weights per input channel on DVE into fp8 pair tiles;
                # the wk@t bias corrections are tiny PE matmuls (PE is idle
                # here anyway).
                tcb = []
                for t in range(CT):
                    tb = stat_pool.tile([P, 1], bf16, tag="tcb", name="tcb")
                    nc.vector.tensor_copy(out=tb, in_=tc_cols[t])
                    tcb.append(tb)
                ws = {}
                for name in ("wkT", "wvT", "wqT"):
                    tiles = [const.tile([P, 2, C], fp8, tag=f"{name}s{cp}",
                                        name=f"{name}s{cp}")
                             for cp in range(CP)]
                    for ci in range(CT):
                        w2 = tiles[ci // 2][:, ci % 2, :]
                        if ci % 2 == 0:
                            nc.vector.tensor_scalar_mul(w2, w_sb[name][ci],
                                                        sc_cols[ci])
                        else:
                            nc.scalar.activation(out=w2, in_=w_sb[name][ci],
                                                 func=ID, scale=sc_cols[ci])
                    ws[name] = tiles

                # bias corrections: bk2[m] = bk[m] + sum_c wk[d,c] t_c
                bias2 = {}
                for name, bcol in (("wkT", "bk"), ("wqT", "bq")):
                    cols2 = []
                    for m in range(CT):
                        tk_ps = pp_sm.tile([P, 1], f32, tag="den", name="den")
                        for ci in range(CT):
                            nc.tensor.matmul(
                                tk_ps,
                                lhsT=w_sb[name][ci][:, m * P:(m + 1) * P],
                                rhs=tcb[ci],
                                start=(ci == 0), stop=(ci == CT - 1))
                        b2 = stat_pool.tile([P, 1], f32, tag=f"b2{name}{m}",
                                            name=f"b2{name}{m}")
                        nc.vector.tensor_scalar(
                            out=b2, in0=tk_ps, scalar1=col_sb[bcol][m],
                            scalar2=None, op0=mybir.AluOpType.add)
                        cols2.append(b2)
                    bias2[name] = cols2
                # v bias row: bvt[c] = bv[c] + sum_c' t_c' wv[c,c'], broadcast
                tv_ps = pp_sm.tile([1, C], f32, tag="den", name="den")
                for ci in range(CT):
                    nc.tensor.matmul(tv_ps, lhsT=tcb[ci], rhs=w_sb["wvT"][ci],
                                     start=(ci == 0), stop=(ci == CT - 1))
                bvt_row = stat_pool.tile([1, C], f32, tag="bvtr", name="bvtr")
                nc.vector.tensor_add(bvt_row, tv_ps, bv_row)
                bvt_ps = pp_av.tile([P, IBS], f32, tag="pav", name="bvtps")
                nc.tensor.matmul(bvt_ps, lhsT=ones_rowf, rhs=bvt_row,
                                 start=True, stop=True)
                bvt_bcast = const.tile([P, C], f32, tag="bvt_bcast",
                                       name="bvt_bcast")
                nc.scalar.activation(out=bvt_bcast, in_=bvt_ps, func=ID)
                worig_cm.__exit__(None, None, None)

                # ---- phase 2: projections straight from fp8 x (DoubleRow) --
                for nch in range(NCH):
                    hsl = slice(nch * IBS, (nch + 1) * IBS)
                    for m in range(CT):
                        ps = pp_mm.tile([P, IBS], f32, tag="mm", name="mm")
                        for cp in range(CP):
                            nc.tensor.matmul(
                                ps,
                                lhsT=ws["wkT"][cp][:, :, m * P:(m + 1) * P],
                                rhs=x2_sb[cp][:, :, hsl],
                                start=(cp == 0), stop=(cp == CP - 1),
                                perf_mode=DR)
                        nc.scalar.activation(
                            out=k2_sb[m // 2][:, m % 2, hsl], in_=ps,
                            func=ID, bias=bias2["wkT"][m], scale=1.0)

                for nch in range(IB):
                    hsl = slice(nch * IBS, (nch + 1) * IBS)
                    for m in range(CT):
                        ps = pp_mm.tile([P, IBS], f32, tag="mm", name="mm")
                        for cp in range(CP):
                            nc.tensor.matmul(
                                ps,
                                lhsT=ws["wqT"][cp][:, :, m * P:(m + 1) * P],
                                rhs=x2_sb[cp][:, :, hsl],
                                start=(cp == 0), stop=(cp == CP - 1),
                                perf_mode=DR)
                        nc.scalar.activation(
                            out=q2_sb[m // 2][:, m % 2, hsl], in_=ps,
                            func=ID, bias=bias2["wqT"][m], scale=1.0)

                # V^T projection; bias-add on DVE drains each PSUM right away
                v2_sb = [v_pool.tile([P, 2, C], fp8, tag="v", name="v")
                         for _ in range(JP)]
                for jt in range(JT):
                    ps = pp_mm.tile([P, IBS], f32, tag="mm", name="mm")
                    for cp in range(CP):
                        nc.tensor.matmul(
                            ps,
                            lhsT=x2_sb[cp][:, :, jt * P:(jt + 1) * P],
                            rhs=ws["wvT"][cp],
                            start=(cp == 0), stop=(cp == CP - 1),
                            perf_mode=DR)
                    nc.vector.tensor_add(v2_sb[jt // 2][:, jt % 2, :],
                                         ps, bvt_bcast)

            # ---- phase 3: attention + output proj + residual ----
            p_pool = ctx.enter_context(tc.tile_pool(name="p", bufs=6))
            xqb_pool = ctx.enter_context(tc.tile_pool(name="xqb", bufs=2 * CT))
            a_pool = ctx.enter_context(tc.tile_pool(name="a", bufs=2 * CT))
            o_pool = ctx.enter_context(tc.tile_pool(name="o", bufs=4))
            sm_pool = ctx.enter_context(tc.tile_pool(name="sm", bufs=2))

            def emit_scores(ib, jp):
                """Scores + exp for key pair-tile jp: returns fp8 [P, 2, IBS]."""
                isl = slice(ib * IBS, (ib + 1) * IBS)
                pt = p_pool.tile([P, 2, IBS], fp8, tag="p", name="p")
                for half in range(2):
                    jt = 2 * jp + half
                    ps = pp_mm.tile([P, IBS], f32, tag="mm", name="mm")
                    for cp in range(CP):
                        nc.tensor.matmul(
                            ps,
                            lhsT=k2_sb[cp][:, :, jt * P:(jt + 1) * P],
                            rhs=q2_sb[cp][:, :, isl],
                            start=(cp == 0), stop=(cp == CP - 1),
                            perf_mode=DR)
                    nc.scalar.activation(out=pt[:, half, :], in_=ps,
                                         func=EXP, scale=SM_SCALE,
                                         bias=shift_col)
                return pt

            # prefetch the residual tiles one block ahead
            xqb_t = [[None] * CT for _ in range(IB)]

            def fetch_xqb(ib):
                isl = slice(ib * IBS, (ib + 1) * IBS)
                for dt_ in range(CT):
                    xt = xqb_pool.tile([P, IBS], f32, tag="xqb", name="xqb")
                    nc.sync.dma_start(out=xt,
                                      in_=xqb_d[dt_ * P:(dt_ + 1) * P, isl])
                    xqb_t[ib][dt_] = xt

            fetch_xqb(0)
            pending = {}
            for ib in range(IB):
                isl = slice(ib * IBS, (ib + 1) * IBS)
                pav = [pp_av.tile([P, IBS], f32, tag="pav", name="pav")
                       for _ in range(CT)]
                # GpSimd accumulates the denominator for jp 0-7, DVE for
                # 8-15; the GpSimd half column-reduces mid-block (jp==13)
                # so only the DVE half is on the block-tail critical path.
                accg = sm_pool.tile([P, IBS], f32, tag="accg", name="accg")
                acc = sm_pool.tile([P, IBS], f32, tag="acc", name="acc")
                den_ps = pp_sm.tile([1, IBS], f32, tag="den", name="den")
                for jp in range(JP):
                    pt = pending.pop((ib, jp), None)
                    if pt is None:
                        pt = emit_scores(ib, jp)
                    eng, at = (nc.gpsimd, accg) if jp < 8 else (nc.vector, acc)
                    if jp in (0, 8):
                        eng.tensor_copy(out=at, in_=pt[:, 0, :])
                    else:
                        eng.tensor_add(at, at, pt[:, 0, :])
                    eng.tensor_add(at, at, pt[:, 1, :])
                    for m in range(CT):
                        nc.tensor.matmul(pav[m],
                                         lhsT=v2_sb[jp][:, :, m * P:(m + 1) * P],
                                         rhs=pt,
                                         start=(jp == 0), stop=(jp == JP - 1),
                                         perf_mode=DR)
                    if jp == 4 and ib + 1 < IB:
                        fetch_xqb(ib + 1)
                    if jp == 13:
                        nc.tensor.matmul(den_ps, lhsT=ones_colf, rhs=accg,
                                         start=True, stop=False)

                # unnormalized attention output -> bf16 (frees pav banks
                # fast). The 1/den scale commutes past the O-projection.
                a_sb = []
                for m in range(CT):
                    at = a_pool.tile([P, IBS], bf16, tag="a", name="a")
                    nc.scalar.activation(out=at, in_=pav[m], func=ID)
                    a_sb.append(at)

                # interleave next-block score lookahead with the denominator
                # tail so neither PE nor ACT drains at the boundary
                if ib + 1 < IB:
                    pending[(ib + 1, 0)] = emit_scores(ib + 1, 0)
                    pending[(ib + 1, 1)] = emit_scores(ib + 1, 1)
                nc.tensor.matmul(den_ps, lhsT=ones_colf, rhs=acc,
                                 start=False, stop=True)
                recip_row = sm_pool.tile([1, IBS], f32, tag="recip_row",
                                         name="recip_row")
                nc.vector.reciprocal(out=recip_row, in_=den_ps)
                # broadcast 1/den across partitions with a K=1 fp32 matmul
                bc_ps = pp_av.tile([P, IBS], f32, tag="pav", name="bcps")
                nc.tensor.matmul(bc_ps, lhsT=ones_rowf, rhs=recip_row,
                                 start=True, stop=True)
                recip_b = sm_pool.tile([P, IBS], f32, tag="recip_b",
                                       name="recip_b")
                nc.scalar.activation(out=recip_b, in_=bc_ps, func=ID)
                if ib + 1 < IB:
                    pending[(ib + 1, 2)] = emit_scores(ib + 1, 2)

                for dt_ in range(CT):
                    po = pp_av.tile([P, IBS], f32, tag="pav", name="po")
                    for m in range(CT):
                        nc.tensor.matmul(
                            po,
                            lhsT=w_sb["woT"][m][:, dt_ * P:(dt_ + 1) * P],
                            rhs=a_sb[m],
                            start=(m == 0), stop=(m == CT - 1))
                    o1 = o_pool.tile([P, IBS], f32, tag="o1", name="o1")
                    nc.vector.tensor_mul(o1, po, recip_b)  # PSUM read: DVE only
                    eng = nc.vector if dt_ < 2 else nc.gpsimd
                    o2 = o_pool.tile([P, IBS], f32, tag="o2", name="o2")
                    eng.tensor_add(o2, o1, xqb_t[ib][dt_])
                    nc.sync.dma_start(out=out_d[dt_ * P:(dt_ + 1) * P, isl],
                                      in_=o2)

    nc.finalize()
    return nc


def _make_consts():
    """Constant (core-independent) input arrays (packed)."""
    ind_a = np.zeros((P, CT * GROUPS), ml_dtypes.bfloat16)
    ind_b = np.zeros((GROUPS, CT * P), ml_dtypes.bfloat16)
    for t in range(CT):
        for p in range(P):
            g = (t * P + p) // GSIZE
            ind_a[p, t * GROUPS + g] = 1.0 / GSIZE
            ind_b[g, t * P + p] = 1.0
    return ind_a, ind_b


def make_in_maps(x, gn_gamma, gn_beta, wq, bq, wk, bk, wv, bv, wo, bo):
    ind_a, ind_b = _make_consts()
    bf = ml_dtypes.bfloat16
    cols = np.stack([np.asarray(a, np.float32) for a in
                     (bq, bk, bv, bo, gn_gamma, gn_beta)], axis=1)
    common = {
        "wqT": np.ascontiguousarray(np.asarray(wq, np.float32).T).astype(bf),
        "wkT": np.ascontiguousarray(np.asarray(wk, np.float32).T).astype(bf),
        "wvT": np.ascontiguousarray(np.asarray(wv, np.float32).T).astype(bf),
        "woT": np.ascontiguousarray(np.asarray(wo, np.float32).T).astype(bf),
        "cols": np.ascontiguousarray(cols),
        "ind_a": ind_a,
        "ind_b": ind_b,
    }
    x = np.asarray(x, np.float32)
    in_maps = []
    for core in range(N_CORES):
        b, half = divmod(core, 2)
        xb = x[b].reshape(C, N)
        xr = np.concatenate(
            [xb[:, half * NQ:(half + 1) * NQ],
             xb[:, (1 - half) * NQ:(2 - half) * NQ]],
            axis=1)
        xqb = xr[:, :NQ] + np.asarray(bo, np.float32).reshape(C, 1)
        in_maps.append({"xs": np.ascontiguousarray(xr).astype(bf),
                        "xqb": np.ascontiguousarray(xqb), **common})
    return in_maps


def gather_out(results):
    out = np.empty((B, C, N), np.float32)
    for core in range(N_CORES):
        b, half = divmod(core, 2)
        out[b][:, half * NQ:(half + 1) * NQ] = results[core]["out"]
    return out.reshape(B, C, H, W)


def get_nc():
    if "nc" not in _cache:
        _cache["nc"] = _build_nc()
    return _cache["nc"]


def kernel(**inputs):
    from concourse.bass_utils import run_bass_kernel_spmd

    nc = get_nc()
    in_maps = make_in_maps(**inputs)
    res = run_bass_kernel_spmd(nc, in_maps, list(range(N_CORES)))
    return gather_out(res.results)


if __name__ == "__main__":
    nc = _build_nc()
    print("built ok:", len(nc.m.functions[0].allocations), "allocations")


# revision 14
# speedup vs baseline: 1.2948x; 1.2948x over previous
"""Trainium2 Bass kernel for AttnBlock (GroupNorm + 1x1-conv QKV self-attention
+ output proj + residual) on x: [4, 512, 64, 64] fp32, distributed over 8
NeuronCores.

Sharding: data-parallel over batch (4) x sequence-parallel over the N=H*W=4096
token axis (2 halves) = 8 cores. Each core receives the full image of its
batch element with the token axis rotated so that its 2048 query tokens come
first; it computes GroupNorm + K/V for all 4096 tokens (duplicated within the
batch pair -- no collectives needed) and Q/attention/output only for its 2048
queries. The host gathers the 8 [512, 2048] outputs back into [4, 512, 64, 64].

All large matmuls run in fp8e4 with MatmulPerfMode.DoubleRow (2 contraction
k-tiles per instruction, ~2x bf16 PE throughput) and fp32 PSUM accumulation;
only the O-projection stays bf16 (its operand, the unnormalized attention
output, exceeds fp8e4's +-240 range). Softmax runs in fp32 (exp on the scalar
engine straight out of PSUM, with a constant -1.5 shift so the fp8 exp output
stays below the 240 saturation point; the shift cancels in the softmax ratio).
Structure:
- x ships once in fp8 pair layout; GroupNorm stats run on DVE bn_stats over
  the same fp8 chunks as they stream in.
- GroupNorm is folded into the projections: wk@(s*x+t) = (wk*s)@x + (wk@t),
  so K/Q/V matmuls consume raw fp8 x tiles; the per-channel scale s lands in
  the fp8 weights (tiny DVE ops) and wk@t lands in the biases (tiny PE
  matmuls). 1/sqrt(var+eps) is computed with a Newton iteration on the
  vector engine so the scalar engine only ever runs Identity/Exp -- a single
  activation-table load for the whole kernel (table-set switches cost ~2.7us).
- All fp8 operands use a channel/key-paired [128, 2, *] layout so each
  DoubleRow matmul consumes two 128-wide contraction tiles at once.
- Scores are computed transposed (S^T = K^T Q per key tile) so softmax and
  the attention@V contraction need no transposes at all.
- The softmax denominator is accumulated on GpSimd for the first half of the
  key tiles and DVE for the second, then column-reduced by two accumulating
  PE matmuls -- the GpSimd half issues mid-block so only the DVE half is on
  the block-boundary critical path. The 1/denominator is applied after the
  O-projection (it commutes with the linear projection).
- Score/exp lookahead into the next query block is interleaved with the
  denominator/reciprocal tail so the PE never drains at block boundaries;
  residual tiles (x+bo) prefetch one block ahead; the final normalize+residual
  ops are split DVE/GpSimd.
"""

import numpy as np
import ml_dtypes

B, C, H, W = 4, 512, 64, 64
N = H * W            # 4096 tokens
NQ = N // 2          # 2048 queries per core
P = 128              # partitions
CT = C // P          # 4 channel tiles
CP = CT // 2         # 2 channel pair-tiles (DoubleRow)
JT = N // P          # 32 key/token tiles
JP = JT // 2         # 16 key pair-tiles (DoubleRow)
IBS = 512            # query block (free dim of score matmuls)
IB = NQ // IBS       # 4 query blocks per core
NCH = N // IBS       # 8 n-chunks for full-N projections
GROUPS = 32
GSIZE = C // GROUPS  # 16 channels per group
EPS = 1e-6
SM_SCALE = float(C) ** -0.5
EXP_SHIFT = -1.5     # exp(s + shift): keeps fp8 exp output < 240 saturation

N_CORES = 8

_cache = {}


def _build_nc():
    import concourse.bass as bass
    import concourse.mybir as mybir
    import concourse.tile as tile
    from concourse import bacc

    f32 = mybir.dt.float32
    bf16 = mybir.dt.bfloat16
    fp8 = mybir.dt.float8e4
    DR = mybir.MatmulPerfMode.DoubleRow
    ID = mybir.ActivationFunctionType.Identity
    EXP = mybir.ActivationFunctionType.Exp

    nc = bacc.Bacc("TRN2")

    xr_d = nc.declare_dram_parameter("xr", [C, N], fp8, isOutput=False)
    w_d = {
        name: nc.declare_dram_parameter(name, [C, C], bf16, isOutput=False)
        for name in ("wqT", "wkT", "wvT", "woT")
    }
    cols_d = nc.declare_dram_parameter("cols", [C, 6], f32, isOutput=False)
    xqb_d = nc.declare_dram_parameter("xqb", [C, NQ], f32, isOutput=False)
    inda_d = nc.declare_dram_parameter("ind_a", [P, CT * GROUPS], bf16, isOutput=False)
    indb_d = nc.declare_dram_parameter("ind_b", [GROUPS, CT * P], bf16, isOutput=False)
    out_d = nc.declare_dram_parameter("out", [C, NQ], f32, isOutput=True)

    with tile.TileContext(nc) as tc:
        from contextlib import ExitStack

        with ExitStack() as ctx:
            const = ctx.enter_context(tc.tile_pool(name="const", bufs=1))
            pp_mm = ctx.enter_context(tc.tile_pool(name="pp_mm", bufs=3, space="PSUM"))
            pp_av = ctx.enter_context(tc.tile_pool(name="pp_av", bufs=4, space="PSUM"))
            pp_sm = ctx.enter_context(tc.tile_pool(name="pp_sm", bufs=1, space="PSUM"))

            # ---- batched small constants (few DMAs; issued after x) ----
            cols_t = [const.tile([P, 6], f32, tag=f"cols{t}", name=f"cols{t}")
                      for t in range(CT)]
            inda_t = const.tile([P, CT * GROUPS], bf16, tag="inda", name="inda")
            indb_t = const.tile([GROUPS, CT * P], bf16, tag="indb", name="indb")
            col_sb = {nm: [cols_t[t][:, i:i + 1] for t in range(CT)]
                      for i, nm in enumerate(("bq", "bk", "bv", "bo",
                                              "gamma", "beta"))}
            inda_sb = [inda_t[:, t * GROUPS:(t + 1) * GROUPS] for t in range(CT)]
            indb_sb = [indb_t[:, t * P:(t + 1) * P] for t in range(CT)]

            ones_colf = const.tile([P, 1], f32, tag="ones_colf", name="ones_colf")
            nc.vector.memset(ones_colf, 1.0)
            ones_rowf = const.tile([1, P], f32, tag="ones_rowf", name="ones_rowf")
            nc.vector.memset(ones_rowf, 1.0)
            shift_col = const.tile([P, 1], f32, tag="shift_col", name="shift_col")
            nc.vector.memset(shift_col, EXP_SHIFT)
            # touch Exp immediately so the one activation-table load (the set
            # holding Exp; Identity is filler in every set) happens at kernel
            # start instead of stalling the first attention block
            warm = const.tile([1, 1], f32, tag="warm", name="warm")
            nc.scalar.activation(out=warm, in_=shift_col[0:1, 0:1], func=EXP)

            stat_pool = ctx.enter_context(tc.tile_pool(name="stat", bufs=4 * CT))

            k_pool = ctx.enter_context(tc.tile_pool(name="k", bufs=CP))
            v_pool = ctx.enter_context(tc.tile_pool(name="v", bufs=JP))
            q_pool = ctx.enter_context(tc.tile_pool(name="q", bufs=CP))
            k2_sb = [k_pool.tile([P, 2, N], fp8, tag="k", name="k")
                     for _ in range(CP)]
            q2_sb = [q_pool.tile([P, 2, NQ], fp8, tag="q", name="q")
                     for _ in range(CP)]

            # ---- phase 1: fp8 x load (2 HW-DGE queues) + GroupNorm stats,
            # all via DVE bn_stats (no ACT Square -> no extra table set) ----
            mv_sb = []
            with tc.tile_pool(name="xr", bufs=CP) as xr_pool:
                x2_sb = [xr_pool.tile([P, 2, N], fp8, tag="xr", name="xr")
                         for _ in range(CP)]
                CHW = N // 4
                # chunk-major load order so early chunks of every tile land
                # first (bn_stats consumes per-tile column chunks in order)
                for ch in range(4):
                    for t in range(CT):
                        eng = nc.sync if (ch * CT + t) % 2 == 0 else nc.scalar
                        csl = slice(ch * CHW, (ch + 1) * CHW)
                        eng.dma_start(out=x2_sb[t // 2][:, t % 2, csl],
                                      in_=xr_d[t * P:(t + 1) * P, csl])
                st_sb = [stat_pool.tile([P, N // 512, 6], f32, tag=f"bnst{t}",
                                        name=f"bnst{t}") for t in range(CT)]
                for ch in range(4):
                    for t in range(CT):
                        for s in range(2 * ch, 2 * ch + 2):
                            nc.vector.bn_stats(
                                out=st_sb[t][:, s, :],
                                in_=x2_sb[t // 2][:, t % 2,
                                                  s * 512:(s + 1) * 512])

                # batched consts + weights + bv now (queues free after x)
                nc.sync.dma_start(out=inda_t, in_=inda_d[:, :])
                nc.sync.dma_start(out=indb_t, in_=indb_d[:, :])
                for t in range(CT):
                    nc.sync.dma_start(out=cols_t[t],
                                      in_=cols_d[t * P:(t + 1) * P, :])
                worig_cm = tc.tile_pool(name="worig", bufs=1)
                worig_pool = worig_cm.__enter__()
                w_sb = {}
                for name in ("wkT", "wqT", "wvT", "woT"):
                    tiles = []
                    for t in range(CT):
                        pool = const if name == "woT" else worig_pool
                        tw = pool.tile([P, C], bf16, tag=f"{name}{t}",
                                       name=f"{name}{t}")
                        nc.sync.dma_start(out=tw,
                                          in_=w_d[name][t * P:(t + 1) * P, :])
                        tiles.append(tw)
                    w_sb[name] = tiles
                bv_row = const.tile([1, C], f32, tag="bv_row", name="bv_row")
                nc.sync.dma_start(
                    out=bv_row,
                    in_=cols_d[:, 2:3].rearrange("c one -> one c"))

                for t in range(CT):
                    mv = stat_pool.tile([P, 2], f32, tag="mv", name="mv")
                    nc.vector.bn_aggr(out=mv, in_=st_sb[t])
                    # mv = [mean, var] -> [mean, E[x^2]]
                    msq = stat_pool.tile([P, 1], f32, tag="msq", name="msq")
                    nc.vector.tensor_mul(msq, mv[:, 0:1], mv[:, 0:1])
                    nc.vector.tensor_add(mv[:, 1:2], mv[:, 1:2], msq)
                    mvb = stat_pool.tile([P, 2], bf16, tag="mvb", name="mvb")
                    nc.vector.tensor_copy(out=mvb, in_=mv)
                    mv_sb.append(mvb)

                # aggregate over channel groups: [32, 2] = [mean_g, E[x^2]_g]
                g_ps = pp_sm.tile([GROUPS, 2], f32, tag="den", name="den")
                for t in range(CT):
                    nc.tensor.matmul(g_ps, lhsT=inda_sb[t], rhs=mv_sb[t],
                                     start=(t == 0), stop=(t == CT - 1))
                g_sb = stat_pool.tile([GROUPS, 2], f32, tag="gsb", name="gsb")
                nc.vector.tensor_copy(out=g_sb, in_=g_ps)
                gm2 = stat_pool.tile([GROUPS, 1], f32, tag="gm2", name="gm2")
                nc.vector.tensor_mul(gm2, g_sb[:, 0:1], g_sb[:, 0:1])
                gvar = stat_pool.tile([GROUPS, 1], f32, tag="gvar", name="gvar")
                nc.vector.tensor_sub(gvar, g_sb[:, 1:2], gm2)
                # ga = 1/sqrt(gvar+EPS) on DVE: y0 = 1.5 - 0.5 v (var ~= 1
                # after GroupNorm-scale inputs), then two Newton steps
                # y <- y(1.5 - 0.5 v y^2). Avoids the ACT Sqrt table set.
                veps = stat_pool.tile([GROUPS, 1], f32, tag="veps", name="veps")
                nc.vector.tensor_scalar(out=veps, in0=gvar, scalar1=EPS,
                                        scalar2=None, op0=mybir.AluOpType.add)
                ga = stat_pool.tile([GROUPS, 1], f32, tag="ga", name="ga")
                nc.vector.tensor_scalar(out=ga, in0=veps, scalar1=-0.5,
                                        scalar2=1.5, op0=mybir.AluOpType.mult,
                                        op1=mybir.AluOpType.add)
                for _ in range(2):
                    yy = stat_pool.tile([GROUPS, 1], f32, tag="yy", name="yy")
                    nc.vector.tensor_mul(yy, ga, ga)
                    nc.vector.tensor_mul(yy, veps, yy)
                    nc.vector.tensor_scalar(out=yy, in0=yy, scalar1=-0.5,
                                            scalar2=1.5,
                                            op0=mybir.AluOpType.mult,
                                            op1=mybir.AluOpType.add)
                    nc.vector.tensor_mul(ga, ga, yy)
                coeffs = stat_pool.tile([GROUPS, 2], bf16, tag="coef", name="coef")
                nc.vector.tensor_copy(out=coeffs[:, 0:1], in_=ga)
                nc.vector.tensor_copy(out=coeffs[:, 1:2], in_=g_sb[:, 0:1])

                # broadcast group coeffs to per-channel scale/shift columns
                sc_cols = []
                tc_cols = []
                for t in range(CT):
                    b_ps = pp_sm.tile([P, 2], f32, tag="den", name="den")
                    nc.tensor.matmul(b_ps, lhsT=indb_sb[t], rhs=coeffs,
                                     start=True, stop=True)
                    bc = stat_pool.tile([P, 2], f32, tag="bc", name="bc")
                    nc.vector.tensor_copy(out=bc, in_=b_ps)
                    s_col = stat_pool.tile([P, 1], f32, tag="scol", name="scol")
                    nc.vector.tensor_mul(s_col, col_sb["gamma"][t], bc[:, 0:1])
                    tmp = stat_pool.tile([P, 1], f32, tag="tmp", name="tmp")
                    nc.vector.tensor_mul(tmp, bc[:, 1:2], s_col)
                    t_col = stat_pool.tile([P, 1], f32, tag="tcol", name="tcol")
                    nc.vector.tensor_sub(t_col, col_sb["beta"][t], tmp)
                    sc_cols.append(s_col)
                    tc_cols.append(t_col)

                # GroupNorm folding: wk@(s*x+t) = (wk*s)@x + wk@t.  Scale the
                # QKV weights per input channel on DVE into fp8 pair tiles;
                # the wk@t bias corrections are tiny PE matmuls (PE is idle
                # here anyway).
                tcb = []
                for t in range(CT):
                    tb = stat_pool.tile([P, 1], bf16, tag="tcb", name="tcb")
                    nc.vector.tensor_copy(out=tb, in_=tc_cols[t])
                    tcb.append(tb)
                ws = {}
                for name in ("wkT", "wvT", "wqT"):
                    tiles = [const.tile([P, 2, C], fp8, tag=f"{name}s{cp}",
                                        name=f"{name}s{cp}")
                             for cp in range(CP)]
                    for ci in range(CT):
                        w2 = tiles[ci // 2][:, ci % 2, :]
                        if ci % 2 == 0:
                            nc.vector.tensor_scalar_mul(w2, w_sb[name][ci],
                                                        sc_cols[ci])
                        else:
                            nc.scalar.activation(out=w2, in_=w_sb[name][ci],
                                                 func=ID, scale=sc_cols[ci])
                    ws[name] = tiles

                # bias corrections: bk2[m] = bk[m] + sum_c wk[d,c] t_c
                bias2 = {}
                for name, bcol in (("wkT", "bk"), ("wqT", "bq")):
                    cols2 = []
                    for m in range(CT):
                        tk_ps = pp_sm.tile([P, 1], f32, tag="den", name="den")
                        for ci in range(CT):
                            nc.tensor.matmul(
                                tk_ps,
                                lhsT=w_sb[name][ci][:, m * P:(m + 1) * P],
                                rhs=tcb[ci],
                                start=(ci == 0), stop=(ci == CT - 1))
                        b2 = stat_pool.tile([P, 1], f32, tag=f"b2{name}{m}",
                                            name=f"b2{name}{m}")
                        nc.vector.tensor_scalar(
                            out=b2, in0=tk_ps, scalar1=col_sb[bcol][m],
                            scalar2=None, op0=mybir.AluOpType.add)
                        cols2.append(b2)
                    bias2[name] = cols2
                # v bias row: bvt[c] = bv[c] + sum_c' t_c' wv[c,c'], broadcast
                tv_ps = pp_sm.tile([1, C], f32, tag="den", name="den")
                for ci in range(CT):
                    nc.tensor.matmul(tv_ps, lhsT=tcb[ci], rhs=w_sb["wvT"][ci],
                                     start=(ci == 0), stop=(ci == CT - 1))
                bvt_row = stat_pool.tile([1, C], f32, tag="bvtr", name="bvtr")
                nc.vector.tensor_add(bvt_row, tv_ps, bv_row)
                bvt_ps = pp_av.tile([P, IBS], f32, tag="pav", name="bvtps")
                nc.tensor.matmul(bvt_ps, lhsT=ones_rowf, rhs=bvt_row,
                                 start=True, stop=True)
                bvt_bcast = const.tile([P, C], f32, tag="bvt_bcast",
                                       name="bvt_bcast")
                nc.scalar.activation(out=bvt_bcast, in_=bvt_ps, func=ID)
                worig_cm.__exit__(None, None, None)

                # ---- phase 2: projections straight from fp8 x (DoubleRow) --
                for nch in range(NCH):
                    hsl = slice(nch * IBS, (nch + 1) * IBS)
                    for m in range(CT):
                        ps = pp_mm.tile([P, IBS], f32, tag="mm", name="mm")
                        for cp in range(CP):
                            nc.tensor.matmul(
                                ps,
                                lhsT=ws["wkT"][cp][:, :, m * P:(m + 1) * P],
                                rhs=x2_sb[cp][:, :, hsl],
                                start=(cp == 0), stop=(cp == CP - 1),
                                perf_mode=DR)
                        nc.scalar.activation(
                            out=k2_sb[m // 2][:, m % 2, hsl], in_=ps,
                            func=ID, bias=bias2["wkT"][m], scale=1.0)

                for nch in range(IB):
                    hsl = slice(nch * IBS, (nch + 1) * IBS)
                    for m in range(CT):
                        ps = pp_mm.tile([P, IBS], f32, tag="mm", name="mm")
                        for cp in range(CP):
                            nc.tensor.matmul(
                                ps,
                                lhsT=ws["wqT"][cp][:, :, m * P:(m + 1) * P],
                                rhs=x2_sb[cp][:, :, hsl],
                                start=(cp == 0), stop=(cp == CP - 1),
                                perf_mode=DR)
                        nc.scalar.activation(
                            out=q2_sb[m // 2][:, m % 2, hsl], in_=ps,
                            func=ID, bias=bias2["wqT"][m], scale=1.0)

                # V^T projection; bias-add on DVE drains each PSUM right away
                v2_sb = [v_pool.tile([P, 2, C], fp8, tag="v", name="v")
                         for _ in range(JP)]
                for jt in range(JT):
                    ps = pp_mm.tile([P, IBS], f32, tag="mm", name="mm")
                    for cp in range(CP):
                        nc.tensor.matmul(
                            ps,
                            lhsT=x2_sb[cp][:, :, jt * P:(jt + 1) * P],
                            rhs=ws["wvT"][cp],
                            start=(cp == 0), stop=(cp == CP - 1),
                            perf_mode=DR)
                    nc.vector.tensor_add(v2_sb[jt // 2][:, jt % 2, :],
                                         ps, bvt_bcast)

            # ---- phase 3: attention + output proj + residual ----
            p_pool = ctx.enter_context(tc.tile_pool(name="p", bufs=6))
            xqb_pool = ctx.enter_context(tc.tile_pool(name="xqb", bufs=2 * CT))
            a_pool = ctx.enter_context(tc.tile_pool(name="a", bufs=2 * CT))
            o_pool = ctx.enter_context(tc.tile_pool(name="o", bufs=4))
            sm_pool = ctx.enter_context(tc.tile_pool(name="sm", bufs=2))

            def emit_scores(ib, jp):
                """Scores + exp for key pair-tile jp: returns fp8 [P, 2, IBS]."""
                isl = slice(ib * IBS, (ib + 1) * IBS)
                pt = p_pool.tile([P, 2, IBS], fp8, tag="p", name="p")
                for half in range(2):
                    jt = 2 * jp + half
                    ps = pp_mm.tile([P, IBS], f32, tag="mm", name="mm")
                    for cp in range(CP):
                        nc.tensor.matmul(
                            ps,
                            lhsT=k2_sb[cp][:, :, jt * P:(jt + 1) * P],
                            rhs=q2_sb[cp][:, :, isl],
                            start=(cp == 0), stop=(cp == CP - 1),
                            perf_mode=DR)
                    nc.scalar.activation(out=pt[:, half, :], in_=ps,
                                         func=EXP, scale=SM_SCALE,
                                         bias=shift_col)
                return pt

            # prefetch the residual tiles one block ahead
            xqb_t = [[None] * CT for _ in range(IB)]

            def fetch_xqb(ib):
                isl = slice(ib * IBS, (ib + 1) * IBS)
                for dt_ in range(CT):
                    xt = xqb_pool.tile([P, IBS], f32, tag="xqb", name="xqb")
                    nc.sync.dma_start(out=xt,
                                      in_=xqb_d[dt_ * P:(dt_ + 1) * P, isl])
                    xqb_t[ib][dt_] = xt

            fetch_xqb(0)
            pending = {}
            for ib in range(IB):
                isl = slice(ib * IBS, (ib + 1) * IBS)
                pav = [pp_av.tile([P, IBS], f32, tag="pav", name="pav")
                       for _ in range(CT)]
                # softmax denominator partials: DVE takes the even key tile
                # of each pair, GpSimd the odd one -- each engine sees one
                # ~0.6/1.1us op per 1.5us of PE work, so neither falls behind
                acc = sm_pool.tile([P, IBS], f32, tag="acc", name="acc")
                accg = sm_pool.tile([P, IBS], f32, tag="accg", name="accg")
                den_ps = pp_sm.tile([1, IBS], f32, tag="den", name="den")
                for jp in range(JP):
                    pt = pending.pop((ib, jp), None)
                    if pt is None:
                        pt = emit_scores(ib, jp)
                    if jp == 0:
                        nc.vector.tensor_copy(out=acc, in_=pt[:, 0, :])
                        nc.gpsimd.tensor_copy(out=accg, in_=pt[:, 1, :])
                    else:
                        nc.vector.tensor_add(acc, acc, pt[:, 0, :])
                        nc.gpsimd.tensor_add(accg, accg, pt[:, 1, :])
                    for m in range(CT):
                        nc.tensor.matmul(pav[m],
                                         lhsT=v2_sb[jp][:, :, m * P:(m + 1) * P],
                                         rhs=pt,
                                         start=(jp == 0), stop=(jp == JP - 1),
                                         perf_mode=DR)
                    if jp == 4 and ib + 1 < IB:
                        fetch_xqb(ib + 1)

                # unnormalized attention output -> bf16 (frees pav banks
                # fast). The 1/den scale commutes past the O-projection.
                a_sb = []
                for m in range(CT):
                    at = a_pool.tile([P, IBS], bf16, tag="a", name="a")
                    nc.scalar.activation(out=at, in_=pav[m], func=ID)
                    a_sb.append(at)

                # interleave next-block score lookahead with the denominator
                # tail so neither PE nor ACT drains at the boundary
                if ib + 1 < IB:
                    pending[(ib + 1, 0)] = emit_scores(ib + 1, 0)
                    pending[(ib + 1, 1)] = emit_scores(ib + 1, 1)
                nc.tensor.matmul(den_ps, lhsT=ones_colf, rhs=acc,
                                 start=True, stop=False)
                nc.tensor.matmul(den_ps, lhsT=ones_colf, rhs=accg,
                                 start=False, stop=True)
                recip_row = sm_pool.tile([1, IBS], f32, tag="recip_row",
                                         name="recip_row")
                nc.vector.reciprocal(out=recip_row, in_=den_ps)
                # broadcast 1/den across partitions with a K=1 fp32 matmul
                bc_ps = pp_av.tile([P, IBS], f32, tag="pav", name="bcps")
                nc.tensor.matmul(bc_ps, lhsT=ones_rowf, rhs=recip_row,
                                 start=True, stop=True)
                recip_b = sm_pool.tile([P, IBS], f32, tag="recip_b",
                                       name="recip_b")
                nc.scalar.activation(out=recip_b, in_=bc_ps, func=ID)
                if ib + 1 < IB:
                    pending[(ib + 1, 2)] = emit_scores(ib + 1, 2)

                for dt_ in range(CT):
                    po = pp_av.tile([P, IBS], f32, tag="pav", name="po")
                    for m in range(CT):
                        nc.tensor.matmul(
                            po,
                            lhsT=w_sb["woT"][m][:, dt_ * P:(dt_ + 1) * P],
                            rhs=a_sb[m],
                            start=(m == 0), stop=(m == CT - 1))
                    o1 = o_pool.tile([P, IBS], f32, tag="o1", name="o1")
                    nc.vector.tensor_mul(o1, po, recip_b)  # PSUM read: DVE only
                    eng = nc.vector if dt_ < 2 else nc.gpsimd
                    o2 = o_pool.tile([P, IBS], f32, tag="o2", name="o2")
                    eng.tensor_add(o2, o1, xqb_t[ib][dt_])
                    nc.sync.dma_start(out=out_d[dt_ * P:(dt_ + 1) * P, isl],
                                      in_=o2)

    nc.finalize()
    return nc


def _make_consts():
    """Constant (core-independent) input arrays (packed)."""
    ind_a = np.zeros((P, CT * GROUPS), ml_dtypes.bfloat16)
    ind_b = np.zeros((GROUPS, CT * P), ml_dtypes.bfloat16)
    for t in range(CT):
        for p in range(P):
            g = (t * P + p) // GSIZE
            ind_a[p, t * GROUPS + g] = 1.0 / GSIZE
            ind_b[g, t * P + p] = 1.0
    return ind_a, ind_b


def make_in_maps(x, gn_gamma, gn_beta, wq, bq, wk, bk, wv, bv, wo, bo):
    ind_a, ind_b = _make_consts()
    bf = ml_dtypes.bfloat16
    f8 = ml_dtypes.float8_e4m3
    cols = np.stack([np.asarray(a, np.float32) for a in
                     (bq, bk, bv, bo, gn_gamma, gn_beta)], axis=1)
    common = {
        "wqT": np.ascontiguousarray(np.asarray(wq, np.float32).T).astype(bf),
        "wkT": np.ascontiguousarray(np.asarray(wk, np.float32).T).astype(bf),
        "wvT": np.ascontiguousarray(np.asarray(wv, np.float32).T).astype(bf),
        "woT": np.ascontiguousarray(np.asarray(wo, np.float32).T).astype(bf),
        "cols": np.ascontiguousarray(cols),
        "ind_a": ind_a,
        "ind_b": ind_b,
    }
    x = np.asarray(x, np.float32)
    in_maps = []
    for core in range(N_CORES):
        b, half = divmod(core, 2)
        xb = x[b].reshape(C, N)
        xr = np.concatenate(
            [xb[:, half * NQ:(half + 1) * NQ],
             xb[:, (1 - half) * NQ:(2 - half) * NQ]],
            axis=1)
        xqb = xr[:, :NQ] + np.asarray(bo, np.float32).reshape(C, 1)
        in_maps.append({"xr": np.ascontiguousarray(xr).astype(f8),
                        "xqb": np.ascontiguousarray(xqb), **common})
    return in_maps


def gather_out(results):
    out = np.empty((B, C, N), np.float32)
    for core in range(N_CORES):
        b, half = divmod(core, 2)
        out[b][:, half * NQ:(half + 1) * NQ] = results[core]["out"]
    return out.reshape(B, C, H, W)


def get_nc():
    if "nc" not in _cache:
        _cache["nc"] = _build_nc()
    return _cache["nc"]


def kernel(**inputs):
    from concourse.bass_utils import run_bass_kernel_spmd

    nc = get_nc()
    in_maps = make_in_maps(**inputs)
    res = run_bass_kernel_spmd(nc, in_maps, list(range(N_CORES)))
    return gather_out(res.results)


if __name__ == "__main__":
    nc = _build_nc()
    print("built ok:", len(nc.m.functions[0].allocations), "allocations")


# revision 22
# speedup vs baseline: 1.4194x; 1.0963x over previous
"""Trainium2 Bass kernel for AttnBlock (GroupNorm + 1x1-conv QKV self-attention
+ output proj + residual) on x: [4, 512, 64, 64] fp32, distributed over 8
NeuronCores.

Sharding: data-parallel over batch (4) x sequence-parallel over the N=H*W=4096
token axis (2 halves) = 8 cores. Each core receives the full image of its
batch element with the token axis rotated so that its 2048 query tokens come
first; it computes GroupNorm + K/V for all 4096 tokens (duplicated within the
batch pair -- no collectives needed) and Q/attention/output only for its 2048
queries. The host gathers the 8 [512, 2048] outputs back into [4, 512, 64, 64].

All large matmuls run in fp8e4 with MatmulPerfMode.DoubleRow (2 contraction
k-tiles per instruction, ~2x bf16 PE throughput) and fp32 PSUM accumulation;
only the O-projection stays bf16 (its operand, the unnormalized attention
output, exceeds fp8e4's +-240 range). Softmax runs in fp32 (exp on the scalar
engine straight out of PSUM, with a constant -1.5 shift so the fp8 exp output
stays below the 240 saturation point; the shift cancels in the softmax ratio).
Structure:
- x ships once in fp8 pair layout; GroupNorm stats run on DVE bn_stats over
  the same fp8 chunks as they stream in.
- GroupNorm is folded into the projections: wk@(s*x+t) = (wk*s)@x + (wk@t),
  so K/Q/V matmuls consume raw fp8 x tiles; the per-channel scale s lands in
  the fp8 weights (tiny DVE ops) and wk@t lands in the biases (tiny PE
  matmuls). 1/sqrt(var+eps) is computed with a Newton iteration on the
  vector engine so the scalar engine only ever runs Identity/Exp -- a single
  activation-table load for the whole kernel (table-set switches cost ~2.7us).
- All fp8 operands use a channel/key-paired [128, 2, *] layout so each
  DoubleRow matmul consumes two 128-wide contraction tiles at once.
- Scores are computed transposed (S^T = K^T Q per key tile) so softmax and
  the attention@V contraction need no transposes at all.
- The softmax denominator is accumulated on GpSimd for the first half of the
  key tiles and DVE for the second, then column-reduced by two accumulating
  PE matmuls -- the GpSimd half issues mid-block so only the DVE half is on
  the block-boundary critical path. The 1/denominator is applied after the
  O-projection (it commutes with the linear projection).
- Score/exp lookahead into the next query block is interleaved with the
  denominator/reciprocal tail so the PE never drains at block boundaries;
  residual tiles (x+bo) prefetch one block ahead; the final normalize+residual
  ops are split DVE/GpSimd.
"""

import numpy as np
import ml_dtypes

B, C, H, W = 4, 512, 64, 64
N = H * W            # 4096 tokens
NQ = N // 2          # 2048 queries per core
P = 128              # partitions
CT = C // P          # 4 channel tiles
CP = CT // 2         # 2 channel pair-tiles (DoubleRow)
JT = N // P          # 32 key/token tiles
JP = JT // 2         # 16 key pair-tiles (DoubleRow)
IBS = 512            # query block (free dim of score matmuls)
IB = NQ // IBS       # 4 query blocks per core
NCH = N // IBS       # 8 n-chunks for full-N projections
GROUPS = 32
GSIZE = C // GROUPS  # 16 channels per group
EPS = 1e-6
SM_SCALE = float(C) ** -0.5
EXP_SHIFT = -1.5     # exp(s + shift): keeps fp8 exp output < 240 saturation

N_CORES = 8

_cache = {}


def _build_nc():
    import concourse.bass as bass
    import concourse.mybir as mybir
    import concourse.tile as tile
    from concourse import bacc

    f32 = mybir.dt.float32
    f32r = mybir.dt.float32r
    bf16 = mybir.dt.bfloat16
    fp8 = mybir.dt.float8e4
    DR = mybir.MatmulPerfMode.DoubleRow
    ID = mybir.ActivationFunctionType.Identity
    EXP = mybir.ActivationFunctionType.Exp

    nc = bacc.Bacc("TRN2")

    xr_d = nc.declare_dram_parameter("xr", [C, N], fp8, isOutput=False)
    w_d = {
        name: nc.declare_dram_parameter(name, [C, C], bf16, isOutput=False)
        for name in ("wqT", "wkT", "wvT", "woT")
    }
    cols_d = nc.declare_dram_parameter("cols", [C, 6], f32, isOutput=False)
    xqb_d = nc.declare_dram_parameter("xqb", [C, NQ], f32, isOutput=False)
    inda_d = nc.declare_dram_parameter("ind_a", [P, CT * GROUPS], bf16, isOutput=False)
    indb_d = nc.declare_dram_parameter("ind_b", [GROUPS, CT * P], bf16, isOutput=False)
    out_d = nc.declare_dram_parameter("out", [C, NQ], f32, isOutput=True)

    with tile.TileContext(nc) as tc:
        from contextlib import ExitStack

        with ExitStack() as ctx:
            const = ctx.enter_context(tc.tile_pool(name="const", bufs=1))
            pp_mm = ctx.enter_context(tc.tile_pool(name="pp_mm", bufs=3, space="PSUM"))
            pp_av = ctx.enter_context(tc.tile_pool(name="pp_av", bufs=4, space="PSUM"))
            pp_sm = ctx.enter_context(tc.tile_pool(name="pp_sm", bufs=1, space="PSUM"))

            # ---- batched small constants (few DMAs; issued after x) ----
            cols_t = [const.tile([P, 6], f32, tag=f"cols{t}", name=f"cols{t}")
                      for t in range(CT)]
            inda_t = const.tile([P, CT * GROUPS], bf16, tag="inda", name="inda")
            indb_t = const.tile([GROUPS, CT * P], bf16, tag="indb", name="indb")
            col_sb = {nm: [cols_t[t][:, i:i + 1] for t in range(CT)]
                      for i, nm in enumerate(("bq", "bk", "bv", "bo",
                                              "gamma", "beta"))}
            inda_sb = [inda_t[:, t * GROUPS:(t + 1) * GROUPS] for t in range(CT)]
            indb_sb = [indb_t[:, t * P:(t + 1) * P] for t in range(CT)]

            ones_colf32 = const.tile([P, 1], f32, tag="ones_colf32", name="ones_colf32")
            nc.vector.memset(ones_colf32, 1.0)
            ones_colf = const.tile([P, 1], f32r, tag="ones_colf", name="ones_colf")
            nc.vector.tensor_copy(out=ones_colf, in_=ones_colf32)
            ones_rowf = const.tile([1, P], f32, tag="ones_rowf", name="ones_rowf")
            nc.vector.memset(ones_rowf, 1.0)
            shift_col = const.tile([P, 1], f32, tag="shift_col", name="shift_col")
            nc.vector.memset(shift_col, EXP_SHIFT)
            # touch Exp immediately so the one activation-table load (the set
            # holding Exp; Identity is filler in every set) happens at kernel
            # start instead of stalling the first attention block
            warm = const.tile([1, 1], f32, tag="warm", name="warm")
            nc.scalar.activation(out=warm, in_=shift_col[0:1, 0:1], func=EXP)

            stat_pool = ctx.enter_context(tc.tile_pool(name="stat", bufs=4 * CT))

            k_pool = ctx.enter_context(tc.tile_pool(name="k", bufs=CP))
            v_pool = ctx.enter_context(tc.tile_pool(name="v", bufs=JP))
            q_pool = ctx.enter_context(tc.tile_pool(name="q", bufs=CP))
            k2_sb = [k_pool.tile([P, 2, N], fp8, tag="k", name="k")
                     for _ in range(CP)]
            q2_sb = [q_pool.tile([P, 2, NQ], fp8, tag="q", name="q")
                     for _ in range(CP)]

            # ---- phase 1: fp8 x load (2 HW-DGE queues) + GroupNorm stats,
            # all via DVE bn_stats (no ACT Square -> no extra table set) ----
            mv_sb = []
            with tc.tile_pool(name="xr", bufs=CP) as xr_pool:
                x2_sb = [xr_pool.tile([P, 2, N], fp8, tag="xr", name="xr")
                         for _ in range(CP)]
                CHW = N // 4
                # chunk-major load order so early chunks of every tile land
                # first (bn_stats consumes per-tile column chunks in order)
                for ch in range(4):
                    for t in range(CT):
                        eng = nc.sync if (ch * CT + t) % 2 == 0 else nc.scalar
                        csl = slice(ch * CHW, (ch + 1) * CHW)
                        eng.dma_start(out=x2_sb[t // 2][:, t % 2, csl],
                                      in_=xr_d[t * P:(t + 1) * P, csl])
                st_sb = [stat_pool.tile([P, N // 512, 6], f32, tag=f"bnst{t}",
                                        name=f"bnst{t}") for t in range(CT)]
                for ch in range(4):
                    for t in range(CT):
                        for s in range(2 * ch, 2 * ch + 2):
                            nc.vector.bn_stats(
                                out=st_sb[t][:, s, :],
                                in_=x2_sb[t // 2][:, t % 2,
                                                  s * 512:(s + 1) * 512])

                # batched consts + weights + bv now (queues free after x)
                nc.sync.dma_start(out=inda_t, in_=inda_d[:, :])
                nc.sync.dma_start(out=indb_t, in_=indb_d[:, :])
                for t in range(CT):
                    nc.sync.dma_start(out=cols_t[t],
                                      in_=cols_d[t * P:(t + 1) * P, :])
                worig_cm = tc.tile_pool(name="worig", bufs=1)
                worig_pool = worig_cm.__enter__()
                w_sb = {}
                for name in ("wkT", "wqT", "wvT", "woT"):
                    tiles = []
                    for t in range(CT):
                        pool = const if name == "woT" else worig_pool
                        tw = pool.tile([P, C], bf16, tag=f"{name}{t}",
                                       name=f"{name}{t}")
                        nc.sync.dma_start(out=tw,
                                          in_=w_d[name][t * P:(t + 1) * P, :])
                        tiles.append(tw)
                    w_sb[name] = tiles
                bv_row = const.tile([1, C], f32, tag="bv_row", name="bv_row")
                nc.sync.dma_start(
                    out=bv_row,
                    in_=cols_d[:, 2:3].rearrange("c one -> one c"))

                for t in range(CT):
                    mv = stat_pool.tile([P, 2], f32, tag="mv", name="mv")
                    nc.vector.bn_aggr(out=mv, in_=st_sb[t])
                    # mv = [mean, var] -> [mean, E[x^2]]
                    msq = stat_pool.tile([P, 1], f32, tag="msq", name="msq")
                    nc.vector.tensor_mul(msq, mv[:, 0:1], mv[:, 0:1])
                    nc.vector.tensor_add(mv[:, 1:2], mv[:, 1:2], msq)
                    mvb = stat_pool.tile([P, 2], bf16, tag="mvb", name="mvb")
                    nc.vector.tensor_copy(out=mvb, in_=mv)
                    mv_sb.append(mvb)

                # aggregate over channel groups: [32, 2] = [mean_g, E[x^2]_g]
                g_ps = pp_sm.tile([GROUPS, 2], f32, tag="den", name="den")
                for t in range(CT):
                    nc.tensor.matmul(g_ps, lhsT=inda_sb[t], rhs=mv_sb[t],
                                     start=(t == 0), stop=(t == CT - 1))
                g_sb = stat_pool.tile([GROUPS, 2], f32, tag="gsb", name="gsb")
                nc.vector.tensor_copy(out=g_sb, in_=g_ps)
                gm2 = stat_pool.tile([GROUPS, 1], f32, tag="gm2", name="gm2")
                nc.vector.tensor_mul(gm2, g_sb[:, 0:1], g_sb[:, 0:1])
                gvar = stat_pool.tile([GROUPS, 1], f32, tag="gvar", name="gvar")
                nc.vector.tensor_sub(gvar, g_sb[:, 1:2], gm2)
                # ga = 1/sqrt(gvar+EPS) on DVE: y0 = 1.5 - 0.5 v (var ~= 1
                # after GroupNorm-scale inputs), then two Newton steps
                # y <- y(1.5 - 0.5 v y^2). Avoids the ACT Sqrt table set.
                veps = stat_pool.tile([GROUPS, 1], f32, tag="veps", name="veps")
                nc.vector.tensor_scalar(out=veps, in0=gvar, scalar1=EPS,
                                        scalar2=None, op0=mybir.AluOpType.add)
                ga = stat_pool.tile([GROUPS, 1], f32, tag="ga", name="ga")
                nc.vector.tensor_scalar(out=ga, in0=veps, scalar1=-0.5,
                                        scalar2=1.5, op0=mybir.AluOpType.mult,
                                        op1=mybir.AluOpType.add)
                for _ in range(2):
                    yy = stat_pool.tile([GROUPS, 1], f32, tag="yy", name="yy")
                    nc.vector.tensor_mul(yy, ga, ga)
                    nc.vector.tensor_mul(yy, veps, yy)
                    nc.vector.tensor_scalar(out=yy, in0=yy, scalar1=-0.5,
                                            scalar2=1.5,
                                            op0=mybir.AluOpType.mult,
                                            op1=mybir.AluOpType.add)
                    nc.vector.tensor_mul(ga, ga, yy)
                coeffs = stat_pool.tile([GROUPS, 2], bf16, tag="coef", name="coef")
                nc.vector.tensor_copy(out=coeffs[:, 0:1], in_=ga)
                nc.vector.tensor_copy(out=coeffs[:, 1:2], in_=g_sb[:, 0:1])

                # broadcast group coeffs to per-channel scale/shift columns
                sc_cols = []
                tc_cols = []
                for t in range(CT):
                    b_ps = pp_sm.tile([P, 2], f32, tag="den", name="den")
                    nc.tensor.matmul(b_ps, lhsT=indb_sb[t], rhs=coeffs,
                                     start=True, stop=True)
                    bc = stat_pool.tile([P, 2], f32, tag="bc", name="bc")
                    nc.vector.tensor_copy(out=bc, in_=b_ps)
                    s_col = stat_pool.tile([P, 1], f32, tag="scol", name="scol")
                    nc.vector.tensor_mul(s_col, col_sb["gamma"][t], bc[:, 0:1])
                    tmp = stat_pool.tile([P, 1], f32, tag="tmp", name="tmp")
                    nc.vector.tensor_mul(tmp, bc[:, 1:2], s_col)
                    t_col = stat_pool.tile([P, 1], f32, tag="tcol", name="tcol")
                    nc.vector.tensor_sub(t_col, col_sb["beta"][t], tmp)
                    sc_cols.append(s_col)
                    tc_cols.append(t_col)

                # GroupNorm folding: wk@(s*x+t) = (wk*s)@x + wk@t.  Scale the
                # QKV weights per input channel on DVE into fp8 pair tiles;
                # the wk@t bias corrections are tiny PE matmuls (PE is idle
                # here anyway).
                tcb = []
                for t in range(CT):
                    tb = stat_pool.tile([P, 1], bf16, tag="tcb", name="tcb")
                    nc.vector.tensor_copy(out=tb, in_=tc_cols[t])
                    tcb.append(tb)
                ws = {}
                for name in ("wkT", "wvT", "wqT"):
                    tiles = [const.tile([P, 2, C], fp8, tag=f"{name}s{cp}",
                                        name=f"{name}s{cp}")
                             for cp in range(CP)]
                    for ci in range(CT):
                        w2 = tiles[ci // 2][:, ci % 2, :]
                        if ci % 2 == 0:
                            nc.vector.tensor_scalar_mul(w2, w_sb[name][ci],
                                                        sc_cols[ci])
                        else:
                            nc.scalar.activation(out=w2, in_=w_sb[name][ci],
                                                 func=ID, scale=sc_cols[ci])
                    ws[name] = tiles

                # bias corrections: bk2[m] = bk[m] + sum_c wk[d,c] t_c
                bias2 = {}
                for name, bcol in (("wkT", "bk"), ("wqT", "bq")):
                    cols2 = []
                    for m in range(CT):
                        tk_ps = pp_sm.tile([P, 1], f32, tag="den", name="den")
                        for ci in range(CT):
                            nc.tensor.matmul(
                                tk_ps,
                                lhsT=w_sb[name][ci][:, m * P:(m + 1) * P],
                                rhs=tcb[ci],
                                start=(ci == 0), stop=(ci == CT - 1))
                        b2 = stat_pool.tile([P, 1], f32, tag=f"b2{name}{m}",
                                            name=f"b2{name}{m}")
                        nc.vector.tensor_scalar(
                            out=b2, in0=tk_ps, scalar1=col_sb[bcol][m],
                            scalar2=None, op0=mybir.AluOpType.add)
                        cols2.append(b2)
                    bias2[name] = cols2
                # v bias row: bvt[c] = bv[c] + sum_c' t_c' wv[c,c'], broadcast
                tv_ps = pp_sm.tile([1, C], f32, tag="den", name="den")
                for ci in range(CT):
                    nc.tensor.matmul(tv_ps, lhsT=tcb[ci], rhs=w_sb["wvT"][ci],
                                     start=(ci == 0), stop=(ci == CT - 1))
                bvt_row = stat_pool.tile([1, C], f32, tag="bvtr", name="bvtr")
                nc.vector.tensor_add(bvt_row, tv_ps, bv_row)
                bvt_ps = pp_av.tile([P, IBS], f32, tag="pav", name="bvtps")
                nc.tensor.matmul(bvt_ps, lhsT=ones_rowf, rhs=bvt_row,
                                 start=True, stop=True)
                bvt_bcast = const.tile([P, C], f32, tag="bvt_bcast",
                                       name="bvt_bcast")
                nc.scalar.activation(out=bvt_bcast, in_=bvt_ps, func=ID)
                worig_cm.__exit__(None, None, None)

                # ---- phase 2: projections straight from fp8 x (DoubleRow) --
                for nch in range(NCH):
                    hsl = slice(nch * IBS, (nch + 1) * IBS)
                    for m in range(CT):
                        ps = pp_mm.tile([P, IBS], f32, tag="mm", name="mm")
                        for cp in range(CP):
                            nc.tensor.matmul(
                                ps,
                                lhsT=ws["wkT"][cp][:, :, m * P:(m + 1) * P],
                                rhs=x2_sb[cp][:, :, hsl],
                                start=(cp == 0), stop=(cp == CP - 1),
                                perf_mode=DR)
                        nc.scalar.activation(
                            out=k2_sb[m // 2][:, m % 2, hsl], in_=ps,
                            func=ID, bias=bias2["wkT"][m], scale=1.0)

                for nch in range(IB):
                    hsl = slice(nch * IBS, (nch + 1) * IBS)
                    for m in range(CT):
                        ps = pp_mm.tile([P, IBS], f32, tag="mm", name="mm")
                        for cp in range(CP):
                            nc.tensor.matmul(
                                ps,
                                lhsT=ws["wqT"][cp][:, :, m * P:(m + 1) * P],
                                rhs=x2_sb[cp][:, :, hsl],
                                start=(cp == 0), stop=(cp == CP - 1),
                                perf_mode=DR)
                        nc.scalar.activation(
                            out=q2_sb[m // 2][:, m % 2, hsl], in_=ps,
                            func=ID, bias=bias2["wqT"][m], scale=1.0)

                # V^T projection; bias-add on DVE drains each PSUM right away
                v2_sb = [v_pool.tile([P, 2, C], fp8, tag="v", name="v")
                         for _ in range(JP)]
                for jt in range(JT):
                    ps = pp_mm.tile([P, IBS], f32, tag="mm", name="mm")
                    for cp in range(CP):
                        nc.tensor.matmul(
                            ps,
                            lhsT=x2_sb[cp][:, :, jt * P:(jt + 1) * P],
                            rhs=ws["wvT"][cp],
                            start=(cp == 0), stop=(cp == CP - 1),
                            perf_mode=DR)
                    nc.vector.tensor_add(v2_sb[jt // 2][:, jt % 2, :],
                                         ps, bvt_bcast)

            # ---- phase 3: attention + output proj + residual ----
            p_pool = ctx.enter_context(tc.tile_pool(name="p", bufs=6))
            xqb_pool = ctx.enter_context(tc.tile_pool(name="xqb", bufs=2 * CT))
            a_pool = ctx.enter_context(tc.tile_pool(name="a", bufs=2 * CT))
            o_pool = ctx.enter_context(tc.tile_pool(name="o", bufs=4))
            sm_pool = ctx.enter_context(tc.tile_pool(name="sm", bufs=2))

            def emit_scores(ib, jp):
                """Scores + exp for key pair-tile jp: returns fp8 [P, 2, IBS]."""
                isl = slice(ib * IBS, (ib + 1) * IBS)
                pt = p_pool.tile([P, 2, IBS], fp8, tag="p", name="p")
                for half in range(2):
                    jt = 2 * jp + half
                    ps = pp_mm.tile([P, IBS], f32, tag="mm", name="mm")
                    for cp in range(CP):
                        nc.tensor.matmul(
                            ps,
                            lhsT=k2_sb[cp][:, :, jt * P:(jt + 1) * P],
                            rhs=q2_sb[cp][:, :, isl],
                            start=(cp == 0), stop=(cp == CP - 1),
                            perf_mode=DR)
                    nc.scalar.activation(out=pt[:, half, :], in_=ps,
                                         func=EXP, scale=SM_SCALE,
                                         bias=shift_col)
                return pt

            # prefetch the residual tiles one block ahead
            xqb_t = [[None] * CT for _ in range(IB)]

            def fetch_xqb(ib):
                isl = slice(ib * IBS, (ib + 1) * IBS)
                for dt_ in range(CT):
                    xt = xqb_pool.tile([P, IBS], f32, tag="xqb", name="xqb")
                    nc.sync.dma_start(out=xt,
                                      in_=xqb_d[dt_ * P:(dt_ + 1) * P, isl])
                    xqb_t[ib][dt_] = xt

            fetch_xqb(0)
            pending = {}
            for ib in range(IB):
                isl = slice(ib * IBS, (ib + 1) * IBS)
                pav = [pp_av.tile([P, IBS], f32, tag="pav", name="pav")
                       for _ in range(CT)]
                # softmax denominator partials: DVE takes the even key tile
                # of each pair, GpSimd the odd one -- each engine sees one
                # ~0.6/1.1us op per 1.5us of PE work, so neither falls behind
                acc = sm_pool.tile([P, IBS], f32r, tag="acc", name="acc")
                accg = sm_pool.tile([P, IBS], f32r, tag="accg", name="accg")
                den_ps = pp_sm.tile([1, IBS], f32, tag="den", name="den")
                for jp in range(JP):
                    pt = pending.pop((ib, jp), None)
                    if pt is None:
                        pt = emit_scores(ib, jp)
                    if jp == 0:
                        nc.vector.tensor_copy(out=acc, in_=pt[:, 0, :])
                        nc.gpsimd.tensor_copy(out=accg, in_=pt[:, 1, :])
                    else:
                        nc.vector.tensor_add(acc, acc, pt[:, 0, :])
                        nc.gpsimd.tensor_add(accg, accg, pt[:, 1, :])
                    for m in range(CT):
                        nc.tensor.matmul(pav[m],
                                         lhsT=v2_sb[jp][:, :, m * P:(m + 1) * P],
                                         rhs=pt,
                                         start=(jp == 0), stop=(jp == JP - 1),
                                         perf_mode=DR)
                    if jp == 4 and ib + 1 < IB:
                        fetch_xqb(ib + 1)

                # unnormalized attention output -> bf16 (frees pav banks
                # fast). The 1/den scale commutes past the O-projection.
                a_sb = []
                for m in range(CT):
                    at = a_pool.tile([P, IBS], bf16, tag="a", name="a")
                    nc.scalar.activation(out=at, in_=pav[m], func=ID)
                    a_sb.append(at)

                # interleave next-block score lookahead with the denominator
                # tail so neither PE nor ACT drains at the boundary
                if ib + 1 < IB:
                    pending[(ib + 1, 0)] = emit_scores(ib + 1, 0)
                    pending[(ib + 1, 1)] = emit_scores(ib + 1, 1)
                # f32r (1-pass) column reduce; ~1e-3 input rounding is far
                # below the fp8 noise floor
                nc.tensor.matmul(den_ps, lhsT=ones_colf, rhs=acc,
                                 start=True, stop=False)
                nc.tensor.matmul(den_ps, lhsT=ones_colf, rhs=accg,
                                 start=False, stop=True)
                recip_row = sm_pool.tile([1, IBS], f32, tag="recip_row",
                                         name="recip_row")
                nc.vector.reciprocal_approx_fast(out=recip_row, in_=den_ps)
                if ib + 1 < IB:
                    pending[(ib + 1, 2)] = emit_scores(ib + 1, 2)

                po_l = []
                for dt_ in range(CT):
                    po = pp_av.tile([P, IBS], f32, tag="pav", name="po")
                    for m in range(CT):
                        nc.tensor.matmul(
                            po,
                            lhsT=w_sb["woT"][m][:, dt_ * P:(dt_ + 1) * P],
                            rhs=a_sb[m],
                            start=(m == 0), stop=(m == CT - 1))
                    po_l.append(po)
                # broadcast 1/den across partitions with a K=1 f32r matmul,
                # emitted after the O-proj so the PE never waits on the recip
                bc_ps = pp_mm.tile([P, IBS], f32, tag="mm", name="bcps")
                nc.tensor.matmul(bc_ps, lhsT=ones_rowf, rhs=recip_row,
                                 start=True, stop=True)
                recip_b = sm_pool.tile([P, IBS], f32, tag="recip_b",
                                       name="recip_b")
                nc.scalar.activation(out=recip_b, in_=bc_ps, func=ID)

                for dt_ in range(CT):
                    o1 = o_pool.tile([P, IBS], f32, tag="o1", name="o1")
                    nc.vector.tensor_mul(o1, po_l[dt_], recip_b)
                    o2 = o_pool.tile([P, IBS], f32, tag="o2", name="o2")
                    nc.vector.tensor_add(o2, o1, xqb_t[ib][dt_])
                    nc.sync.dma_start(out=out_d[dt_ * P:(dt_ + 1) * P, isl],
                                      in_=o2)

    nc.finalize()
    return nc


def _make_consts():
    """Constant (core-independent) input arrays (packed)."""
    ind_a = np.zeros((P, CT * GROUPS), ml_dtypes.bfloat16)
    ind_b = np.zeros((GROUPS, CT * P), ml_dtypes.bfloat16)
    for t in range(CT):
        for p in range(P):
            g = (t * P + p) // GSIZE
            ind_a[p, t * GROUPS + g] = 1.0 / GSIZE
            ind_b[g, t * P + p] = 1.0
    return ind_a, ind_b


def make_in_maps(x, gn_gamma, gn_beta, wq, bq, wk, bk, wv, bv, wo, bo):
    ind_a, ind_b = _make_consts()
    bf = ml_dtypes.bfloat16
    f8 = ml_dtypes.float8_e4m3
    cols = np.stack([np.asarray(a, np.float32) for a in
                     (bq, bk, bv, bo, gn_gamma, gn_beta)], axis=1)
    common = {
        "wqT": np.ascontiguousarray(np.asarray(wq, np.float32).T).astype(bf),
        "wkT": np.ascontiguousarray(np.asarray(wk, np.float32).T).astype(bf),
        "wvT": np.ascontiguousarray(np.asarray(wv, np.float32).T).astype(bf),
        "woT": np.ascontiguousarray(np.asarray(wo, np.float32).T).astype(bf),
        "cols": np.ascontiguousarray(cols),
        "ind_a": ind_a,
        "ind_b": ind_b,
    }
    x = np.asarray(x, np.float32)
    in_maps = []
    for core in range(N_CORES):
        b, half = divmod(core, 2)
        xb = x[b].reshape(C, N)
        xr = np.concatenate(
            [xb[:, half * NQ:(half + 1) * NQ],
             xb[:, (1 - half) * NQ:(2 - half) * NQ]],
            axis=1)
        xqb = xr[:, :NQ] + np.asarray(bo, np.float32).reshape(C, 1)
        in_maps.append({"xr": np.ascontiguousarray(xr).astype(f8),
                        "xqb": np.ascontiguousarray(xqb), **common})
    return in_maps


def gather_out(results):
    out = np.empty((B, C, N), np.float32)
    for core in range(N_CORES):
        b, half = divmod(core, 2)
        out[b][:, half * NQ:(half + 1) * NQ] = results[core]["out"]
    return out.reshape(B, C, H, W)


def get_nc():
    if "nc" not in _cache:
        _cache["nc"] = _build_nc()
    return _cache["nc"]


def kernel(**inputs):
    from concourse.bass_utils import run_bass_kernel_spmd

    nc = get_nc()
    in_maps = make_in_maps(**inputs)
    res = run_bass_kernel_spmd(nc, in_maps, list(range(N_CORES)))
    return gather_out(res.results)


if __name__ == "__main__":
    nc = _build_nc()
    print("built ok:", len(nc.m.functions[0].allocations), "allocations")
